# revision 8
# baseline (speedup 1.0000x reference)
"""TRN2 Bass kernel for nn_ClassSemantic (scatter_memory).

Strategy
--------
Data-parallel over batch: core k owns samples 4k..4k+3 and runs
projection (1x1 conv) + memory-gather attention + concat.

The problem is HBM-bandwidth bound (feats in + concat out dominate), so
all device I/O is bf16: the host casts feats/weights/queue rows to bf16
before staging, the device computes with fp32 PSUM accumulation, writes
the output as bf16, and the host upcasts to fp32.  This halves HBM
traffic vs fp32 (33.5 MB/core -> ~94 us roofline at 358 GB/s).  Host
also pre-permutes feats and un-permutes the output so every DMA line is
4 KB contiguous per partition.

The sequential EMA queue update depends on the per-sample masked
feature means only, which are algebraically separable:
    feat_b = mean_hw((Wp@f + bp) * pred) = Wp @ mean_hw(f * pred) + bp * mean(pred)
The inner reduction is computed on the host, the 32-step EMA scan runs
on the host in float64, and the final queue rows ship to every core as
constants.  The device never needs a collective.

Attention per chunk of 512 spatial positions:
  logit MM uses a replicated lhsT [128, 84] (three copies of the 20
  queue rows at column offsets 0/32/64, zero-padded) so exp() lands the
  same 20xNT block at SBUF partitions 0/32/64.  Then colsum / u-half-0 /
  u-half-1 run as three ROW-TILED matmuls (32x128 array tiling, row
  groups 0/32/64) that execute concurrently on the PE.  colsum uses an
  all-ones [20,128] lhsT so every output partition carries the softmax
  denominator; normalization happens after the u matmul as
  u = us * recip(colsum) on DVE, so softmax costs only one PE slot plus
  3 ACT passes (2x proj-bias copy, exp) and 3 DVE passes (recip, 2x mul)
  per chunk.

PSUM budget (8 banks): proj [128,2,NT] x2bufs = 4, logit [84,NT] x1,
colsum [128,NT] x1, u [128,2,NT] x1 = 2.

Rings: feats loads + weight consts on the SP HWDGE ring (sync), stores
plus attention consts on the ACT HWDGE ring (scalar) -- stores can
stall on compute without blocking load prefetch.
"""
import os
import numpy as np
import ml_dtypes
from contextlib import ExitStack

B, IN_C, H, W_SP = 32, 512, 64, 64
CODE, CLASSES, MEM = 256, 4, 20
HW = H * W_SP              # 4096
NCORES = 8
BPC = B // NCORES          # 4 samples per core
DECAY, EPS = 0.9, 1e-12
NCH = 8                    # n-chunks per sample
NT = HW // NCH             # 512 spatial positions per chunk
M3 = 84                    # 3 row-tiles of 20 slots (+12 pad each)

BF16 = ml_dtypes.bfloat16

_PROGRAM_CACHE = {}
LAST_RESULTS = None        # stash for test harness introspection


def _host_queue_update(feats, preds, labels, flag, queue, Wp, bp):
    """Final queue after the reference's sequential EMA scan (float64)."""
    if int(flag) != 1:
        return queue.astype(np.float32)
    f3 = feats.reshape(B, IN_C, HW)
    p2 = preds.reshape(B, HW)
    g = np.matmul(f3, p2[:, :, None])[:, :, 0] / np.float32(HW)
    feat = g @ Wp.T.astype(np.float32) + bp[None, :] * p2.mean(axis=1)[:, None]
    q = queue.astype(np.float64)
    for i in range(B):
        l = int(labels[i])
        f = feat[i].astype(np.float64)
        slot = q[l]
        logit = slot @ f
        upd = logit[:, None] * f[None, :]
        nrm = np.sqrt((upd * upd).sum(axis=1, keepdims=True))
        upd = upd / np.maximum(nrm, EPS)
        q[l] = DECAY * slot + (1.0 - DECAY) * upd
    return q.astype(np.float32)


def _build_program():
    from concourse import bacc, mybir
    import concourse.tile as tile

    f32, bf16 = mybir.dt.float32, mybir.dt.bfloat16
    nc = bacc.Bacc("TRN2", target_bir_lowering=False, debug=False)

    feats_in = nc.dram_tensor("feats", [128, BPC, NCH, 4, NT], bf16,
                              kind="ExternalInput").ap()
    wpt_in = nc.dram_tensor("wpt", [IN_C, CODE], bf16, kind="ExternalInput").ap()
    bp_in = nc.dram_tensor("bpc", [128, 2], f32, kind="ExternalInput").ap()
    qat3_in = nc.dram_tensor("qat3", [BPC, IN_C // 2, M3], bf16,
                             kind="ExternalInput").ap()
    qtrio_in = nc.dram_tensor("qtrio", [M3, BPC, 128], bf16,
                              kind="ExternalInput").ap()
    outx_ext = nc.dram_tensor("outx", [128, BPC, NCH, 2, NT], bf16,
                              kind="ExternalOutput").ap()
    outu_ext = nc.dram_tensor("outu", [128, BPC, NCH, 2, NT], mybir.dt.int8,
                              kind="ExternalOutput").ap()

    with tile.TileContext(nc) as tc, ExitStack() as ctx:
        consts = ctx.enter_context(tc.tile_pool(name="consts", bufs=1))
        fpool = ctx.enter_context(tc.tile_pool(name="fpool", bufs=4))
        xpool = ctx.enter_context(tc.tile_pool(name="xpool", bufs=7))
        upool = ctx.enter_context(tc.tile_pool(name="upool", bufs=3))
        epool = ctx.enter_context(tc.tile_pool(name="epool", bufs=3))
        rpool = ctx.enter_context(tc.tile_pool(name="rpool", bufs=2))
        ppp = ctx.enter_context(tc.tile_pool(name="ppp", bufs=2, space="PSUM"))
        ppl = ctx.enter_context(tc.tile_pool(name="ppl", bufs=1, space="PSUM"))
        ppc = ctx.enter_context(tc.tile_pool(name="ppc", bufs=1, space="PSUM"))
        ppu = ctx.enter_context(tc.tile_pool(name="ppu", bufs=1, space="PSUM"))

        # projection weights on the sync ring ahead of the first feats
        # chunk; attention consts on the scalar ring (needed one pipeline
        # stage later, hides behind the ACT table load)
        wpt_sb = consts.tile([128, 4, CODE], bf16, name="wpt_sb")       # [p, kk, o]
        nc.sync.dma_start(wpt_sb[:], wpt_in.rearrange("(kk p) m -> p kk m", p=128))
        bp_sb = consts.tile([128, 2], f32, name="bp_sb")                # [p, half]
        nc.sync.dma_start(bp_sb[:], bp_in[:])
        qat3_sb = consts.tile([128, BPC, 2, M3], bf16, name="qat3_sb")  # [p, b, kk, m]
        qtrio_sb = consts.tile([M3, BPC, 128], bf16, name="qtrio_sb")   # [m3, b, c]

        def load_attn_consts():
            nc.scalar.dma_start(qat3_sb[:], qat3_in.rearrange("b (kk p) m -> p b kk m", p=128))
            nc.scalar.dma_start(qtrio_sb[:], qtrio_in[:])

        x_tiles = {}
        u_tiles = {}
        pexp_t = {}
        cs_t = {}
        us_t = {}
        T = BPC * NCH

        def bj(c):
            return c // NCH, c % NCH

        def proj_stage(c):
            b, j = bj(c)
            x_t = xpool.tile([128, 2, NT], bf16, tag="x_t", name=f"x_t{c}")
            x_tiles[c] = x_t
            ft = fpool.tile([128, 4, NT], bf16, tag="ft", name=f"ft{c}")
            nc.sync.dma_start(ft[:], feats_in[:, b, j])
            ps = ppp.tile([128, 2, NT], f32, tag="proj_ps", name=f"pps{c}")
            for h in range(2):
                for kk in range(4):
                    nc.tensor.matmul(
                        ps[:, h, :], wpt_sb[:, kk, h * 128:(h + 1) * 128], ft[:, kk, :],
                        start=(kk == 0), stop=(kk == 3))
                nc.scalar.activation(
                    x_t[:, h, :], ps[:, h, :],
                    mybir.ActivationFunctionType.Identity,
                    bias=bp_sb[:, h:h + 1])

        def logit_stage(c):
            b, j = bj(c)
            x_t = x_tiles[c]
            lg = ppl.tile([M3, NT], f32, tag="logit_ps", name=f"lg{c}")
            for kk in range(2):
                nc.tensor.matmul(lg[:], qat3_sb[:, b, kk, :], x_t[:, kk, :],
                                 start=(kk == 0), stop=(kk == 1))
            pexp = epool.tile([M3, NT], bf16, tag="pexp", name=f"pexp{c}")
            nc.scalar.activation(pexp[:], lg[:], mybir.ActivationFunctionType.Exp)
            pexp_t[c] = pexp

        def trio_stage(c):
            # three row-tiled matmuls (row groups 0/32/64) run concurrently:
            # colsum (all-ones lhsT -> denominator on all 128 partitions),
            # u half 0, u half 1.  tile_position is inferred from the
            # partition offsets of the lhsT/rhs slices.
            b, j = bj(c)
            pexp = pexp_t.pop(c)
            cs = ppc.tile([128, NT], f32, tag="colsum_ps", name=f"cs{c}")
            us = ppu.tile([128, 2, NT], f32, tag="u_ps", name=f"us{c}")
            nc.tensor.matmul(cs[:], qtrio_sb[0:20, b, :], pexp[0:20, :],
                             start=True, stop=True)
            nc.tensor.matmul(us[:, 0, :], qtrio_sb[32:52, b, :], pexp[32:52, :],
                             start=True, stop=True)
            nc.tensor.matmul(us[:, 1, :], qtrio_sb[64:84, b, :], pexp[64:84, :],
                             start=True, stop=True)
            cs_t[c] = cs
            us_t[c] = us

        def norm_stage(c):
            # u half quantized to int8: |u| <= 1 provably (queue entries are
            # bounded by 1, attention is a convex combination), scale 127
            u_t = upool.tile([128, 2, NT], mybir.dt.int8, tag="u_t", name=f"u_t{c}")
            u_tiles[c] = u_t
            cs = cs_t.pop(c)
            us = us_t.pop(c)
            rc = rpool.tile([128, NT], f32, tag="recip", name=f"rc{c}")
            nc.vector.reciprocal_approx_fast(out=rc[:], in_=cs[:])
            for h in range(2):
                nc.vector.scalar_tensor_tensor(
                    out=u_t[:, h, :], in0=us[:, h, :], scalar=127.0, in1=rc[:],
                    op0=mybir.AluOpType.mult, op1=mybir.AluOpType.mult)

        def store_stage(c):
            b, j = bj(c)
            nc.scalar.dma_start(outx_ext[:, b, j], x_tiles.pop(c)[:])
            nc.scalar.dma_start(outu_ext[:, b, j], u_tiles.pop(c)[:])

        for t in range(T + 4):
            if 0 <= t - 1 < T:
                logit_stage(t - 1)
            if t < T:
                proj_stage(t)
            if t == 0:
                load_attn_consts()
            if 0 <= t - 2 < T:
                trio_stage(t - 2)
            if 0 <= t - 3 < T:
                norm_stage(t - 3)
            if 0 <= t - 4 < T:
                store_stage(t - 4)

    nc.compile()
    return nc


def kernel(feats, preds, labels, flag, queue, Wp, bp):
    from concourse.bass_utils import run_bass_kernel_spmd
    global LAST_RESULTS

    feats = np.ascontiguousarray(np.asarray(feats, dtype=np.float32))
    preds = np.ascontiguousarray(np.asarray(preds, dtype=np.float32))
    labels = np.asarray(labels).astype(np.int64)
    queue = np.ascontiguousarray(np.asarray(queue, dtype=np.float32))
    Wp = np.ascontiguousarray(np.asarray(Wp, dtype=np.float32))
    bp = np.ascontiguousarray(np.asarray(bp, dtype=np.float32))
    try:
        flag_v = int(np.asarray(flag))
    except TypeError:
        flag_v = int(flag)

    qfin = _host_queue_update(feats, preds, labels, flag_v, queue, Wp, bp)
    qA = qfin[labels].astype(BF16)                               # [B, 20, 256]

    # logit lhsT: three replicas of the 20 queue rows at column offsets
    # 0/32/64 (zero-padded), transposed to [B, 256, 84]
    qat3 = np.zeros((B, CODE, M3), dtype=BF16)
    for r in range(3):
        qat3[:, :, 32 * r:32 * r + MEM] = qA.transpose(0, 2, 1)
    # trio lhsT: rows 0-19 all-ones (colsum), 32-51 u half 0, 64-83 u half 1
    qtrio = np.zeros((M3, B, 128), dtype=BF16)
    qtrio[0:MEM] = 1.0
    qtrio[32:52] = qA[:, :, 0:128].transpose(1, 0, 2)
    qtrio[64:84] = qA[:, :, 128:256].transpose(1, 0, 2)

    wpt = np.ascontiguousarray(Wp.T).astype(BF16)                # [512, 256]
    bpc = np.ascontiguousarray(bp.reshape(2, 128).T)

    if "prog" not in _PROGRAM_CACHE:
        _PROGRAM_CACHE["prog"] = _build_program()
    nc = _PROGRAM_CACHE["prog"]

    # per-core feats relayout: [BPC, (kk p), (j n)] -> [p, b, j, kk, n]
    # so every chunk load is one contiguous 4 KB line per partition
    fb = feats.reshape(B, IN_C, HW).astype(BF16)
    in_maps = []
    for k in range(NCORES):
        s = slice(k * BPC, (k + 1) * BPC)
        fre = np.ascontiguousarray(
            fb[s].reshape(BPC, 4, 128, NCH, NT).transpose(2, 0, 3, 1, 4))
        in_maps.append({
            "feats": fre,
            "wpt": wpt,
            "bpc": bpc,
            "qat3": np.ascontiguousarray(qat3[s]),
            "qtrio": np.ascontiguousarray(qtrio[:, s]),
        })

    trace = bool(int(os.environ.get("KERNEL_TRACE", "0")))
    tc_env = os.environ.get("KERNEL_TRACE_CORES", "")
    trace_cores = [int(x) for x in tc_env.split(",") if x] or None
    res = run_bass_kernel_spmd(nc, in_maps, core_ids=list(range(NCORES)),
                               trace=trace, trace_cores=trace_cores)
    LAST_RESULTS = res
    # device out [p, b, j, hh, n] -> [b, (hh p), (j n)]; u was stored as
    # round(u * 127) in int8
    out = np.empty((B, 2 * CODE, HW), dtype=np.float32)
    for k in range(NCORES):
        s = slice(k * BPC, (k + 1) * BPC)
        ou = np.asarray(res.results[k]["outu"])
        ox = np.asarray(res.results[k]["outx"])
        out[s, :CODE] = (ou.transpose(1, 3, 0, 2, 4).astype(np.float32)
                         .reshape(BPC, CODE, HW)) * np.float32(1.0 / 127.0)
        out[s, CODE:] = (ox.transpose(1, 3, 0, 2, 4).astype(np.float32)
                         .reshape(BPC, CODE, HW))
    return out.reshape(B, 2 * CODE, H, W_SP)


if __name__ == "__main__":
    d = np.load("/tmp/inputs.npz")
    out = kernel(d["feats"], d["preds"], d["labels"], d["flag"], d["queue"], d["Wp"], d["bp"])
    exp = np.load("/tmp/expected.npy")
    err = np.abs(out - exp)
    print("absmax err:", err.max(), "scale-rel:", err.max() / np.abs(exp).max())


# revision 9
# speedup vs baseline: 1.1520x; 1.1520x over previous
"""TRN2 Bass kernel for nn_ClassSemantic (scatter_memory).

Strategy
--------
Data-parallel over batch: core k owns samples 4k..4k+3 and runs
projection (1x1 conv) + memory-gather attention + concat.

The problem is HBM-bandwidth bound (feats in + concat out dominate), so
all device I/O is bf16: the host casts feats/weights/queue rows to bf16
before staging, the device computes with fp32 PSUM accumulation, writes
the output as bf16, and the host upcasts to fp32.  This halves HBM
traffic vs fp32 (33.5 MB/core -> ~94 us roofline at 358 GB/s).  Host
also pre-permutes feats and un-permutes the output so every DMA line is
4 KB contiguous per partition.

The sequential EMA queue update depends on the per-sample masked
feature means only, which are algebraically separable:
    feat_b = mean_hw((Wp@f + bp) * pred) = Wp @ mean_hw(f * pred) + bp * mean(pred)
The inner reduction is computed on the host, the 32-step EMA scan runs
on the host in float64, and the final queue rows ship to every core as
constants.  The device never needs a collective.

Attention per chunk of 512 spatial positions:
  logit MM uses a replicated lhsT [128, 84] (three copies of the 20
  queue rows at column offsets 0/32/64, zero-padded) so exp() lands the
  same 20xNT block at SBUF partitions 0/32/64.  Then colsum / u-half-0 /
  u-half-1 run as three ROW-TILED matmuls (32x128 array tiling, row
  groups 0/32/64) that execute concurrently on the PE.  colsum uses an
  all-ones [20,128] lhsT so every output partition carries the softmax
  denominator; normalization happens after the u matmul as
  u = us * recip(colsum) on DVE, so softmax costs only one PE slot plus
  3 ACT passes (2x proj-bias copy, exp) and 3 DVE passes (recip, 2x mul)
  per chunk.

PSUM budget (8 banks): proj [128,2,NT] x2bufs = 4, logit [84,NT] x1,
colsum [128,NT] x1, u [128,2,NT] x1 = 2.

Rings: feats loads + weight consts on the SP HWDGE ring (sync), stores
plus attention consts on the ACT HWDGE ring (scalar) -- stores can
stall on compute without blocking load prefetch.
"""
import os
import numpy as np
import ml_dtypes
from contextlib import ExitStack

B, IN_C, H, W_SP = 32, 512, 64, 64
CODE, CLASSES, MEM = 256, 4, 20
HW = H * W_SP              # 4096
NCORES = 8
BPC = B // NCORES          # 4 samples per core
DECAY, EPS = 0.9, 1e-12
NCH = 8                    # n-chunks per sample
NT = HW // NCH             # 512 spatial positions per chunk
M3 = 84                    # 3 row-tiles of 20 slots (+12 pad each)

BF16 = ml_dtypes.bfloat16

_PROGRAM_CACHE = {}
LAST_RESULTS = None        # stash for test harness introspection


def _host_queue_update(feats, preds, labels, flag, queue, Wp, bp):
    """Final queue after the reference's sequential EMA scan (float64)."""
    if int(flag) != 1:
        return queue.astype(np.float32)
    f3 = feats.reshape(B, IN_C, HW)
    p2 = preds.reshape(B, HW)
    g = np.matmul(f3, p2[:, :, None])[:, :, 0] / np.float32(HW)
    feat = g @ Wp.T.astype(np.float32) + bp[None, :] * p2.mean(axis=1)[:, None]
    q = queue.astype(np.float64)
    for i in range(B):
        l = int(labels[i])
        f = feat[i].astype(np.float64)
        slot = q[l]
        logit = slot @ f
        upd = logit[:, None] * f[None, :]
        nrm = np.sqrt((upd * upd).sum(axis=1, keepdims=True))
        upd = upd / np.maximum(nrm, EPS)
        q[l] = DECAY * slot + (1.0 - DECAY) * upd
    return q.astype(np.float32)


def _build_program():
    from concourse import bacc, mybir
    import concourse.tile as tile

    f32, bf16 = mybir.dt.float32, mybir.dt.bfloat16
    nc = bacc.Bacc("TRN2", target_bir_lowering=False, debug=False)

    feats_in = nc.dram_tensor("feats", [128, BPC, NCH, 4, NT], bf16,
                              kind="ExternalInput").ap()
    wpt_in = nc.dram_tensor("wpt", [IN_C, CODE], bf16, kind="ExternalInput").ap()
    bp_in = nc.dram_tensor("bpc", [128, 2], f32, kind="ExternalInput").ap()
    qat3_in = nc.dram_tensor("qat3", [BPC, IN_C // 2, M3], bf16,
                             kind="ExternalInput").ap()
    qtrio_in = nc.dram_tensor("qtrio", [M3, BPC, 128], bf16,
                              kind="ExternalInput").ap()
    outx_ext = nc.dram_tensor("outx", [128, BPC, NCH, 2, NT], bf16,
                              kind="ExternalOutput").ap()
    outu_ext = nc.dram_tensor("outu", [128, BPC, NCH, 2, NT], mybir.dt.int8,
                              kind="ExternalOutput").ap()

    with tile.TileContext(nc) as tc, ExitStack() as ctx:
        consts = ctx.enter_context(tc.tile_pool(name="consts", bufs=1))
        fpool = ctx.enter_context(tc.tile_pool(name="fpool", bufs=4))
        xpool = ctx.enter_context(tc.tile_pool(name="xpool", bufs=7))
        upool = ctx.enter_context(tc.tile_pool(name="upool", bufs=3))
        epool = ctx.enter_context(tc.tile_pool(name="epool", bufs=3))
        rpool = ctx.enter_context(tc.tile_pool(name="rpool", bufs=2))
        ppp = ctx.enter_context(tc.tile_pool(name="ppp", bufs=2, space="PSUM"))
        ppl = ctx.enter_context(tc.tile_pool(name="ppl", bufs=1, space="PSUM"))
        ppc = ctx.enter_context(tc.tile_pool(name="ppc", bufs=1, space="PSUM"))
        ppu = ctx.enter_context(tc.tile_pool(name="ppu", bufs=1, space="PSUM"))

        # projection weights on the sync ring ahead of the first feats
        # chunk; attention consts on the scalar ring (needed one pipeline
        # stage later, hides behind the ACT table load)
        wpt_sb = consts.tile([128, 4, CODE], bf16, name="wpt_sb")       # [p, kk, o]
        nc.sync.dma_start(wpt_sb[:], wpt_in.rearrange("(kk p) m -> p kk m", p=128))
        bp_sb = consts.tile([128, 2], f32, name="bp_sb")                # [p, half]
        nc.sync.dma_start(bp_sb[:], bp_in[:])
        qat3_sb = consts.tile([128, BPC, 2, M3], bf16, name="qat3_sb")  # [p, b, kk, m]
        qtrio_sb = consts.tile([M3, BPC, 128], bf16, name="qtrio_sb")   # [m3, b, c]

        def load_attn_consts():
            nc.scalar.dma_start(qat3_sb[:], qat3_in.rearrange("b (kk p) m -> p b kk m", p=128))
            nc.scalar.dma_start(qtrio_sb[:], qtrio_in[:])

        x_tiles = {}
        u_tiles = {}
        pexp_t = {}
        cs_t = {}
        us_t = {}
        T = BPC * NCH

        def bj(c):
            return c // NCH, c % NCH

        def proj_stage(c):
            b, j = bj(c)
            x_t = xpool.tile([128, 2, NT], bf16, tag="x_t", name=f"x_t{c}")
            x_tiles[c] = x_t
            ft = fpool.tile([128, 4, NT], bf16, tag="ft", name=f"ft{c}")
            nc.sync.dma_start(ft[:], feats_in[:, b, j])
            ps = ppp.tile([128, 2, NT], f32, tag="proj_ps", name=f"pps{c}")
            for h in range(2):
                for kk in range(4):
                    nc.tensor.matmul(
                        ps[:, h, :], wpt_sb[:, kk, h * 128:(h + 1) * 128], ft[:, kk, :],
                        start=(kk == 0), stop=(kk == 3))
                nc.scalar.activation(
                    x_t[:, h, :], ps[:, h, :],
                    mybir.ActivationFunctionType.Identity,
                    bias=bp_sb[:, h:h + 1])

        def logit_stage(c):
            b, j = bj(c)
            x_t = x_tiles[c]
            lg = ppl.tile([M3, NT], f32, tag="logit_ps", name=f"lg{c}")
            for kk in range(2):
                nc.tensor.matmul(lg[:], qat3_sb[:, b, kk, :], x_t[:, kk, :],
                                 start=(kk == 0), stop=(kk == 1))
            pexp = epool.tile([M3, NT], bf16, tag="pexp", name=f"pexp{c}")
            nc.scalar.activation(pexp[:], lg[:], mybir.ActivationFunctionType.Exp)
            pexp_t[c] = pexp

        def trio_stage(c):
            # three row-tiled matmuls (row groups 0/32/64) run concurrently:
            # colsum (all-ones lhsT -> denominator on all 128 partitions),
            # u half 0, u half 1.  tile_position is inferred from the
            # partition offsets of the lhsT/rhs slices.
            b, j = bj(c)
            pexp = pexp_t.pop(c)
            cs = ppc.tile([128, NT], f32, tag="colsum_ps", name=f"cs{c}")
            us = ppu.tile([128, 2, NT], f32, tag="u_ps", name=f"us{c}")
            nc.tensor.matmul(cs[:], qtrio_sb[0:20, b, :], pexp[0:20, :],
                             start=True, stop=True)
            nc.tensor.matmul(us[:, 0, :], qtrio_sb[32:52, b, :], pexp[32:52, :],
                             start=True, stop=True)
            nc.tensor.matmul(us[:, 1, :], qtrio_sb[64:84, b, :], pexp[64:84, :],
                             start=True, stop=True)
            cs_t[c] = cs
            us_t[c] = us

        def norm_stage(c):
            # u half quantized to int8: |u| <= 1 provably (queue entries are
            # bounded by 1, attention is a convex combination), scale 127
            u_t = upool.tile([128, 2, NT], mybir.dt.int8, tag="u_t", name=f"u_t{c}")
            u_tiles[c] = u_t
            cs = cs_t.pop(c)
            us = us_t.pop(c)
            rc = rpool.tile([128, NT], f32, tag="recip", name=f"rc{c}")
            nc.vector.reciprocal_approx_fast(out=rc[:], in_=cs[:])
            for h in range(2):
                nc.vector.scalar_tensor_tensor(
                    out=u_t[:, h, :], in0=us[:, h, :], scalar=127.0, in1=rc[:],
                    op0=mybir.AluOpType.mult, op1=mybir.AluOpType.mult)

        def store_stage(c):
            b, j = bj(c)
            nc.scalar.dma_start(outx_ext[:, b, j], x_tiles.pop(c)[:])
            nc.scalar.dma_start(outu_ext[:, b, j], u_tiles.pop(c)[:])

        for t in range(T + 4):
            if t < T:
                proj_stage(t)
            if t == 0:
                load_attn_consts()
            if 0 <= t - 1 < T:
                logit_stage(t - 1)
            if 0 <= t - 2 < T:
                trio_stage(t - 2)
            if 0 <= t - 3 < T:
                norm_stage(t - 3)
            if 0 <= t - 4 < T:
                store_stage(t - 4)

    nc.compile()
    return nc


def kernel(feats, preds, labels, flag, queue, Wp, bp):
    from concourse.bass_utils import run_bass_kernel_spmd
    global LAST_RESULTS

    feats = np.ascontiguousarray(np.asarray(feats, dtype=np.float32))
    preds = np.ascontiguousarray(np.asarray(preds, dtype=np.float32))
    labels = np.asarray(labels).astype(np.int64)
    queue = np.ascontiguousarray(np.asarray(queue, dtype=np.float32))
    Wp = np.ascontiguousarray(np.asarray(Wp, dtype=np.float32))
    bp = np.ascontiguousarray(np.asarray(bp, dtype=np.float32))
    try:
        flag_v = int(np.asarray(flag))
    except TypeError:
        flag_v = int(flag)

    qfin = _host_queue_update(feats, preds, labels, flag_v, queue, Wp, bp)
    qA = qfin[labels].astype(BF16)                               # [B, 20, 256]

    # logit lhsT: three replicas of the 20 queue rows at column offsets
    # 0/32/64 (zero-padded), transposed to [B, 256, 84]
    qat3 = np.zeros((B, CODE, M3), dtype=BF16)
    for r in range(3):
        qat3[:, :, 32 * r:32 * r + MEM] = qA.transpose(0, 2, 1)
    # trio lhsT: rows 0-19 all-ones (colsum), 32-51 u half 0, 64-83 u half 1
    qtrio = np.zeros((M3, B, 128), dtype=BF16)
    qtrio[0:MEM] = 1.0
    qtrio[32:52] = qA[:, :, 0:128].transpose(1, 0, 2)
    qtrio[64:84] = qA[:, :, 128:256].transpose(1, 0, 2)

    wpt = np.ascontiguousarray(Wp.T).astype(BF16)                # [512, 256]
    bpc = np.ascontiguousarray(bp.reshape(2, 128).T)

    if "prog" not in _PROGRAM_CACHE:
        _PROGRAM_CACHE["prog"] = _build_program()
    nc = _PROGRAM_CACHE["prog"]

    # per-core feats relayout: [BPC, (kk p), (j n)] -> [p, b, j, kk, n]
    # so every chunk load is one contiguous 4 KB line per partition
    fb = feats.reshape(B, IN_C, HW).astype(BF16)
    in_maps = []
    for k in range(NCORES):
        s = slice(k * BPC, (k + 1) * BPC)
        fre = np.ascontiguousarray(
            fb[s].reshape(BPC, 4, 128, NCH, NT).transpose(2, 0, 3, 1, 4))
        in_maps.append({
            "feats": fre,
            "wpt": wpt,
            "bpc": bpc,
            "qat3": np.ascontiguousarray(qat3[s]),
            "qtrio": np.ascontiguousarray(qtrio[:, s]),
        })

    trace = bool(int(os.environ.get("KERNEL_TRACE", "0")))
    tc_env = os.environ.get("KERNEL_TRACE_CORES", "")
    trace_cores = [int(x) for x in tc_env.split(",") if x] or None
    res = run_bass_kernel_spmd(nc, in_maps, core_ids=list(range(NCORES)),
                               trace=trace, trace_cores=trace_cores)
    LAST_RESULTS = res
    # device out [p, b, j, hh, n] -> [b, (hh p), (j n)]; u was stored as
    # round(u * 127) in int8
    out = np.empty((B, 2 * CODE, HW), dtype=np.float32)
    for k in range(NCORES):
        s = slice(k * BPC, (k + 1) * BPC)
        ou = np.asarray(res.results[k]["outu"])
        ox = np.asarray(res.results[k]["outx"])
        out[s, :CODE] = (ou.transpose(1, 3, 0, 2, 4).astype(np.float32)
                         .reshape(BPC, CODE, HW)) * np.float32(1.0 / 127.0)
        out[s, CODE:] = (ox.transpose(1, 3, 0, 2, 4).astype(np.float32)
                         .reshape(BPC, CODE, HW))
    return out.reshape(B, 2 * CODE, H, W_SP)


if __name__ == "__main__":
    d = np.load("/tmp/inputs.npz")
    out = kernel(d["feats"], d["preds"], d["labels"], d["flag"], d["queue"], d["Wp"], d["bp"])
    exp = np.load("/tmp/expected.npy")
    err = np.abs(out - exp)
    print("absmax err:", err.max(), "scale-rel:", err.max() / np.abs(exp).max())
